# revision 26
# baseline (speedup 1.0000x reference)
"""Bayesian linear layer on 8 TRN2 NeuronCores.

Computes  out = x @ (mu + softplus(rho) * eps_w).T + (bmu + softplus(brho) * eps_b)
for x [16384, 4096], weights [4096, 4096].

Sharding: pure tensor-parallel, 8-way split of out_features. Each core computes
the full-height [16384, 512] fp32 output shard:
  - the host pre-transposes everything to k-major layout, so every device DMA
    is a plain contiguous load: x ships as 16 super-tile slabs [4096 (k),
    1024 (n)] fp16; the weight shard ships as one packed [4096, 1536] fp16
    tensor with 3KB rows [rho | mu | eps] (one DMA per k-block -- 1KB-row
    loads only sustain ~18GB/s per SDMA engine vs ~26 for 3KB rows, and
    phase 1 is DMA-bandwidth-paced).
  - W^T is generated on-device directly in [k, o] layout into 32 resident
    [128, 512] fp16 k-block tiles: softplus via Exp + Ln(x+1) on ACT (single
    batched table set, table pre-warmed by a dummy Exp at t=0), then
    w = mu + sp*eps as two fp16 DVE ops. No DRAM staging, no transpose DMAs.
  - prep is emitted interleaved with the first super-tile's x slabs so the PE
    starts after block 0 (~12us) and consumes k-blocks in prep-completion
    order (j-outer in super-tile 0); from super-tile 1 on it runs gap-free at
    the N=512 fp16 roofline (215.8 ns/matmul, 55.25us per super-tile).
  - matmuls are fp16, N=512, fp32 PSUM accumulation over 32 k-blocks into all
    8 PSUM banks (one per 128-row sub-tile); the host-computed bias is added
    during the PSUM->SBUF copy on DVE.
  - the next super-tile's kq0/kq1 x slabs are prefetched ahead of the current
    out stores in the (strict FIFO) SP ring, so store-drain latency never
    delays the x feed.
All DMAs stay on the SP HWDGE ring: splitting across the SP+ACT rings
corrupts results on this stack (completion tracking assumes one ring).
Measured: ~906us HW exec (vs 874us pure-matmul roofline); run-to-run +0-10%
from chip-level power/thermal throttling (PE drops below 2.4GHz).
"""

import numpy as np

import bass_rust as _bass_rust
import concourse.bacc as bacc
import concourse.tile as tile
from concourse import mybir
from concourse import bass_utils
from concourse.hw_specs import get_activation_tables


class _Bacc(bacc.Bacc):
    """Bacc whose activation-table placement resolves Exp and Ln to the one
    table set containing both (natural_log_exp_and_others), instead of
    thrashing between per-function sets (one 1.3us ACT_TABLE_LOAD per
    ACTIVATE).  List order/indices are preserved -- act_func_set_id is the
    index into act_info.json -- only the membership used for matching is
    restricted."""

    def insert_act_table_loads(self):
        tables = list(get_activation_tables(self.m.arch).items())
        AF = mybir.ActivationFunctionType
        filtered = []
        for name, funcs in tables:
            if name != "natural_log_exp_and_others":
                funcs = funcs - {AF.Exp, AF.Ln}
            filtered.append((name, funcs))
        _bass_rust.insert_act_table_loads(self, filtered)


N, IN_F, OUT_F = 16384, 4096, 4096
N_CORES = 8
OS = OUT_F // N_CORES            # 512 out cols per core
KB = IN_F // 128                 # 32 k-blocks
NB = 1024                        # rows per super-tile
NSUP = N // NB                   # 16 super-tiles
NKQ = 4                          # k-quarters (x slab groups)
KQ = KB // NKQ                   # 8 k-blocks per quarter
SUBS = NB // 128                 # 8 psum sub-tiles per super-tile

FP32 = mybir.dt.float32
F16 = mybir.dt.float16


def _build_nc():
    nc = _Bacc("TRN2", target_bir_lowering=False, debug=False)

    xt = nc.dram_tensor("xt", [NSUP * IN_F, NB], F16, kind="ExternalInput").ap()
    # per k-block row: [rho | mu | eps] packed -> one 3KB-row DMA per block
    wcat = nc.dram_tensor("wcat", [IN_F, 3 * OS], F16, kind="ExternalInput").ap()
    bias = nc.dram_tensor("bias", [128, OS], FP32, kind="ExternalInput").ap()
    out = nc.dram_tensor("out", [N, OS], FP32, kind="ExternalOutput").ap()

    AF = mybir.ActivationFunctionType

    with tile.TileContext(nc) as tc:
        with (
            tc.tile_pool(name="wt", bufs=1) as wt_pool,
            tc.tile_pool(name="bias", bufs=1) as bias_pool,
            tc.tile_pool(name="prep_in", bufs=3) as prep_in,
            tc.tile_pool(name="prep_w", bufs=2) as prep_w,
            tc.tile_pool(name="xt", bufs=1) as xt_pool,
            tc.tile_pool(name="outp", bufs=8) as out_pool,
            tc.tile_pool(name="psum", bufs=1, space="PSUM") as psum_pool,
        ):
            # ACT table prewarm: a dummy Exp on a zeroed tile, emitted first,
            # pulls the 1.3us ACT_TABLE_LOAD into the initial DMA-latency
            # window instead of the first prep block's critical path.
            # (A PE pre-warm with junk matmuls was tried and reverted: the
            # cold-clock matmuls already hide inside s0's DMA-paced stalls.)
            warm = prep_w.tile([128, 8], F16, tag="warm", bufs=1)
            nc.vector.memset(warm[:], 0.0)
            nc.scalar.activation(warm[:], warm[:], AF.Exp)

            wts = [wt_pool.tile([128, OS], F16, tag=f"wt{ib}",
                                name=f"wt{ib}") for ib in range(KB)]

            def xt_panel(s, kq, emit_dma=True):
                xtt = xt_pool.tile([128, KQ * NB], F16, tag=f"kq{kq}",
                                   name=f"xt_s{s}_k{kq}", bufs=2)
                if emit_dma:
                    for j in range(KQ):
                        ib = kq * KQ + j
                        row = s * IN_F + ib * 128
                        nc.sync.dma_start(xtt[:, j * NB:(j + 1) * NB],
                                          xt[row:row + 128, :])
                return xtt

            # ---- prep interleaved with super-tile 0's x slabs: per k-block,
            # ship its x slab, then rho -> softplus on ACT -> w = mu + sp*eps
            # on DVE into the resident wts tile. PE consumption (8 matmuls =
            # 1.7us per block) and DMA supply (640KB = 1.8us per block) stay
            # in lockstep, so the first super-tile runs DMA-paced and the
            # matmul pipeline starts ~13us in (NEFF preamble ~6us + first
            # block's DMA receipt + softplus chain).
            xq0 = [xt_panel(0, kq, emit_dma=False) for kq in range(NKQ)]
            bias_t = None
            for ib in range(KB):
                kq, j = divmod(ib, KQ)
                win = prep_in.tile([128, 3 * OS], F16, tag=f"w{ib % 2}",
                                   name=f"win_{ib}")
                nc.sync.dma_start(win[:], wcat[ib * 128:(ib + 1) * 128, :])
                row = 0 * IN_F + ib * 128
                nc.sync.dma_start(xq0[kq][:, j * NB:(j + 1) * NB],
                                  xt[row:row + 128, :])
                r = win[:, 0:OS]
                nc.scalar.activation(r, r, AF.Exp)
                nc.scalar.activation(r, r, AF.Ln, bias=1.0)
                t = prep_w.tile([128, OS], F16, tag="t")
                nc.vector.tensor_mul(t[:], r, win[:, 2 * OS:3 * OS])
                nc.vector.tensor_add(wts[ib][:], t[:], win[:, OS:2 * OS])
                if ib == 7:
                    # bias (precomputed on host) -- off the critical head,
                    # needed only by the first drain (~55us).
                    bias_t = bias_pool.tile([128, OS], FP32, tag="bias")
                    nc.sync.dma_start(bias_t[:], bias[:])

            # ---- main loop. The next super-tile's kq0/kq1 x slabs are
            # emitted BEFORE this super-tile's out stores: the SP HWDGE ring
            # is strict FIFO, and a store stalls the sequencer until its DVE
            # drain completes -- prefetches enqueued after it would start a
            # whole drain-latency late (measured ~2-3us PE gaps at every kq
            # boundary of the next super-tile).
            xtq = xq0
            for s in range(NSUP):
                psq = [psum_pool.tile([128, OS], FP32, tag=f"ps{sub}",
                                      name=f"ps_{s}_{sub}")
                       for sub in range(SUBS)]
                # s=0 runs j-outer so k-blocks (and their x slabs) are
                # consumed in strict prep-arrival order -- phase 1 is
                # DMA-paced and sub-outer would demand all 8 blocks of a
                # quarter within the first 8 matmuls. s>=1 runs sub-outer so
                # the per-bank stop matmuls spread across the last quarter
                # (bunched stops would serialize all 8 drains after the last
                # matmul and stall the next super-tile's PSUM reuse).
                if s == 0:
                    order = [(sub, j) for j in range(KQ) for sub in range(SUBS)]
                else:
                    order = [(sub, j) for sub in range(SUBS) for j in range(KQ)]
                for kq in range(NKQ):
                    for sub, j in order:
                        ib = kq * KQ + j
                        xs = xtq[kq][:, j * NB + sub * 128:
                                     j * NB + (sub + 1) * 128]
                        nc.tensor.matmul(
                            psq[sub][:], xs, wts[ib][:],
                            start=(ib == 0), stop=(ib == KB - 1))
                if s + 1 < NSUP:
                    # all four panel groups double-buffered and prefetched
                    # ahead of this super-tile's stores: no WAR gate, and the
                    # stores (absorbed by the 8-deep out pool) can lag.
                    xtq = [xt_panel(s + 1, kq) for kq in range(NKQ)]
                for sub in range(SUBS):
                    ot = out_pool.tile([128, OS], FP32, tag="ot",
                                       name=f"ot_{s}_{sub}")
                    nc.vector.tensor_add(ot[:], psq[sub][:], bias_t[:])
                    row = (s * SUBS + sub) * 128
                    nc.sync.dma_start(out[row:row + 128, :], ot[:])

    nc.compile()
    return nc


_NC = None


def _get_nc():
    global _NC
    if _NC is None:
        _NC = _build_nc()
    return _NC


def kernel(x, weight_mu, weight_rho, bias_mu, bias_rho, eps_w, eps_b,
           _trace=False, _trace_kwargs=None):
    x = np.asarray(x, dtype=np.float32)
    weight_mu = np.asarray(weight_mu, dtype=np.float32)
    weight_rho = np.asarray(weight_rho, dtype=np.float32)
    bias_mu = np.asarray(bias_mu, dtype=np.float32)
    bias_rho = np.asarray(bias_rho, dtype=np.float32)
    eps_w = np.asarray(eps_w, dtype=np.float32)
    eps_b = np.asarray(eps_b, dtype=np.float32)

    nc = _get_nc()

    # k-major super-tile slabs: [16, 4096 (k), 1024 (n)] -> [65536, 1024]
    xb = x.astype(np.float16)
    xtv = np.ascontiguousarray(
        xb.reshape(NSUP, NB, IN_F).transpose(0, 2, 1)).reshape(NSUP * IN_F, NB)

    bias_full = bias_mu + np.log1p(np.exp(bias_rho)) * eps_b

    in_maps = []
    for c in range(N_CORES):
        osl = slice(c * OS, (c + 1) * OS)
        wc = np.concatenate([weight_rho[osl].astype(np.float16).T,
                             weight_mu[osl].astype(np.float16).T,
                             eps_w[osl].astype(np.float16).T], axis=1)
        in_maps.append({
            "xt": xtv,
            "wcat": np.ascontiguousarray(wc),
            "bias": np.ascontiguousarray(
                np.broadcast_to(bias_full[osl], (128, OS))),
        })

    kwargs = {}
    if _trace:
        kwargs["trace"] = True
        if _trace_kwargs:
            kwargs.update(_trace_kwargs)
    res = bass_utils.run_bass_kernel_spmd(
        nc, in_maps, core_ids=list(range(N_CORES)), **kwargs)

    out = np.empty((N, OUT_F), np.float32)
    for c in range(N_CORES):
        out[:, c * OS:(c + 1) * OS] = res.results[c]["out"]
    if _trace:
        return out, res
    return out


# revision 28
# speedup vs baseline: 1.0148x; 1.0148x over previous
"""Bayesian linear layer on 8 TRN2 NeuronCores.

Computes  out = x @ (mu + softplus(rho) * eps_w).T + (bmu + softplus(brho) * eps_b)
for x [16384, 4096], weights [4096, 4096].

Sharding: pure tensor-parallel, 8-way split of out_features. Each core computes
the full-height [16384, 512] fp32 output shard:
  - the host pre-transposes everything to k-major layout, so every device DMA
    is a plain contiguous load: x ships as 16 super-tile slabs [4096 (k),
    1024 (n)] fp16; the weight shard ships as one packed [4096, 1536] fp16
    tensor with 3KB rows [rho | mu | eps] (one DMA per k-block -- 1KB-row
    loads only sustain ~18GB/s per SDMA engine vs ~26 for 3KB rows, and
    phase 1 is DMA-bandwidth-paced).
  - W^T is generated on-device directly in [k, o] layout into 32 resident
    [128, 512] fp16 k-block tiles: softplus via Exp + Ln(x+1) on ACT (single
    batched table set, table pre-warmed by a dummy Exp at t=0), then
    w = mu + sp*eps as two fp16 DVE ops. No DRAM staging, no transpose DMAs.
  - prep is emitted interleaved with the first super-tile's x slabs so the PE
    starts after block 0 (~12us) and consumes k-blocks in prep-completion
    order (j-outer in super-tile 0); from super-tile 1 on it runs gap-free at
    the N=512 fp16 roofline (215.8 ns/matmul, 55.25us per super-tile).
  - matmuls are fp16, N=512, fp32 PSUM accumulation over 32 k-blocks into all
    8 PSUM banks (one per 128-row sub-tile); the host-computed bias is added
    during the PSUM->SBUF copy on DVE.
  - the next super-tile's kq0/kq1 x slabs are prefetched ahead of the current
    out stores in the (strict FIFO) SP ring, so store-drain latency never
    delays the x feed.
All DMAs stay on the SP HWDGE ring: splitting across the SP+ACT rings
corrupts results on this stack (completion tracking assumes one ring).
Measured: ~906us HW exec (vs 874us pure-matmul roofline); run-to-run +0-10%
from chip-level power/thermal throttling (PE drops below 2.4GHz).
"""

import numpy as np

import bass_rust as _bass_rust
import concourse.bacc as bacc
import concourse.tile as tile
from concourse import mybir
from concourse import bass_utils
from concourse.hw_specs import get_activation_tables


class _Bacc(bacc.Bacc):
    """Bacc whose activation-table placement resolves Exp and Ln to the one
    table set containing both (natural_log_exp_and_others), instead of
    thrashing between per-function sets (one 1.3us ACT_TABLE_LOAD per
    ACTIVATE).  List order/indices are preserved -- act_func_set_id is the
    index into act_info.json -- only the membership used for matching is
    restricted."""

    def insert_act_table_loads(self):
        tables = list(get_activation_tables(self.m.arch).items())
        AF = mybir.ActivationFunctionType
        filtered = []
        for name, funcs in tables:
            if name != "natural_log_exp_and_others":
                funcs = funcs - {AF.Exp, AF.Ln}
            filtered.append((name, funcs))
        _bass_rust.insert_act_table_loads(self, filtered)


N, IN_F, OUT_F = 16384, 4096, 4096
N_CORES = 8
OS = OUT_F // N_CORES            # 512 out cols per core
KB = IN_F // 128                 # 32 k-blocks
NB = 1024                        # rows per super-tile
NSUP = N // NB                   # 16 super-tiles
NKQ = 4                          # k-quarters (x slab groups)
KQ = KB // NKQ                   # 8 k-blocks per quarter
SUBS = NB // 128                 # 8 psum sub-tiles per super-tile

FP32 = mybir.dt.float32
F16 = mybir.dt.float16


def _build_nc():
    nc = _Bacc("TRN2", target_bir_lowering=False, debug=False)

    xt = nc.dram_tensor("xt", [NSUP * IN_F, NB], F16, kind="ExternalInput").ap()
    # per k-block row: [rho | mu | eps] packed -> one 3KB-row DMA per block
    wcat = nc.dram_tensor("wcat", [IN_F, 3 * OS], F16, kind="ExternalInput").ap()
    bias = nc.dram_tensor("bias", [128, OS], FP32, kind="ExternalInput").ap()
    out = nc.dram_tensor("out", [N, OS], FP32, kind="ExternalOutput").ap()

    AF = mybir.ActivationFunctionType

    with tile.TileContext(nc) as tc:
        with (
            tc.tile_pool(name="wt", bufs=1) as wt_pool,
            tc.tile_pool(name="bias", bufs=1) as bias_pool,
            tc.tile_pool(name="prep_in", bufs=3) as prep_in,
            tc.tile_pool(name="prep_w", bufs=2) as prep_w,
            tc.tile_pool(name="xt", bufs=1) as xt_pool,
            tc.tile_pool(name="outp", bufs=8) as out_pool,
            tc.tile_pool(name="psum", bufs=1, space="PSUM") as psum_pool,
        ):
            # ACT table prewarm: a dummy Exp on a zeroed tile, emitted first,
            # pulls the 1.3us ACT_TABLE_LOAD into the initial DMA-latency
            # window instead of the first prep block's critical path.
            # (A PE pre-warm with junk matmuls was tried and reverted: the
            # cold-clock matmuls already hide inside s0's DMA-paced stalls.)
            warm = prep_w.tile([128, 8], F16, tag="warm", bufs=1)
            nc.vector.memset(warm[:], 0.0)
            nc.scalar.activation(warm[:], warm[:], AF.Exp)

            wts = [wt_pool.tile([128, OS], F16, tag=f"wt{ib}",
                                name=f"wt{ib}") for ib in range(KB)]

            def xt_panel(s, kq, emit_dma=True):
                xtt = xt_pool.tile([128, KQ * NB], F16, tag=f"kq{kq}",
                                   name=f"xt_s{s}_k{kq}",
                                   bufs=2 if kq < 2 else 1)
                if emit_dma:
                    for j in range(KQ):
                        ib = kq * KQ + j
                        row = s * IN_F + ib * 128
                        nc.sync.dma_start(xtt[:, j * NB:(j + 1) * NB],
                                          xt[row:row + 128, :])
                return xtt

            # ---- prep interleaved with super-tile 0's x slabs: per k-block,
            # ship its x slab, then rho -> softplus on ACT -> w = mu + sp*eps
            # on DVE into the resident wts tile. PE consumption (8 matmuls =
            # 1.7us per block) and DMA supply (640KB = 1.8us per block) stay
            # in lockstep, so the first super-tile runs DMA-paced and the
            # matmul pipeline starts ~13us in (NEFF preamble ~6us + first
            # block's DMA receipt + softplus chain).
            xq0 = [xt_panel(0, kq, emit_dma=False) for kq in range(NKQ)]
            bias_t = None
            for ib in range(KB):
                kq, j = divmod(ib, KQ)
                win = prep_in.tile([128, 3 * OS], F16, tag=f"w{ib % 2}",
                                   name=f"win_{ib}")
                nc.sync.dma_start(win[:], wcat[ib * 128:(ib + 1) * 128, :])
                row = 0 * IN_F + ib * 128
                nc.sync.dma_start(xq0[kq][:, j * NB:(j + 1) * NB],
                                  xt[row:row + 128, :])
                r = win[:, 0:OS]
                nc.scalar.activation(r, r, AF.Exp)
                nc.scalar.activation(r, r, AF.Ln, bias=1.0)
                t = prep_w.tile([128, OS], F16, tag="t")
                nc.vector.tensor_mul(t[:], r, win[:, 2 * OS:3 * OS])
                nc.vector.tensor_add(wts[ib][:], t[:], win[:, OS:2 * OS])
                if ib == 7:
                    # bias (precomputed on host) -- off the critical head,
                    # needed only by the first drain (~55us).
                    bias_t = bias_pool.tile([128, OS], FP32, tag="bias")
                    nc.sync.dma_start(bias_t[:], bias[:])

            # ---- main loop. The next super-tile's kq0/kq1 x slabs are
            # emitted BEFORE this super-tile's out stores: the SP HWDGE ring
            # is strict FIFO, and a store stalls the sequencer until its DVE
            # drain completes -- prefetches enqueued after it would start a
            # whole drain-latency late (measured ~2-3us PE gaps at every kq
            # boundary of the next super-tile).
            xtq = xq0
            for s in range(NSUP):
                psq = [psum_pool.tile([128, OS], FP32, tag=f"ps{sub}",
                                      name=f"ps_{s}_{sub}")
                       for sub in range(SUBS)]
                # s=0 runs j-outer so k-blocks (and their x slabs) are
                # consumed in strict prep-arrival order -- phase 1 is
                # DMA-paced and sub-outer would demand all 8 blocks of a
                # quarter within the first 8 matmuls. s>=1 runs sub-outer so
                # the per-bank stop matmuls spread across the last quarter
                # (bunched stops would serialize all 8 drains after the last
                # matmul and stall the next super-tile's PSUM reuse).
                if s == 0:
                    order = [(sub, j) for j in range(KQ) for sub in range(SUBS)]
                else:
                    order = [(sub, j) for sub in range(SUBS) for j in range(KQ)]
                for kq in range(NKQ):
                    for sub, j in order:
                        ib = kq * KQ + j
                        xs = xtq[kq][:, j * NB + sub * 128:
                                     j * NB + (sub + 1) * 128]
                        nc.tensor.matmul(
                            psq[sub][:], xs, wts[ib][:],
                            start=(ib == 0), stop=(ib == KB - 1))
                nxt = None
                if s + 1 < NSUP:
                    nxt = [xt_panel(s + 1, kq) for kq in (0, 1)]
                for sub in range(SUBS):
                    ot = out_pool.tile([128, OS], FP32, tag="ot",
                                       name=f"ot_{s}_{sub}")
                    nc.vector.tensor_add(ot[:], psq[sub][:], bias_t[:])
                    row = (s * SUBS + sub) * 128
                    nc.sync.dma_start(out[row:row + 128, :], ot[:])
                if s + 1 < NSUP:
                    nxt += [xt_panel(s + 1, kq) for kq in (2, 3)]
                    xtq = nxt

    nc.compile()
    return nc


_NC = None


def _get_nc():
    global _NC
    if _NC is None:
        _NC = _build_nc()
    return _NC


def kernel(x, weight_mu, weight_rho, bias_mu, bias_rho, eps_w, eps_b,
           _trace=False, _trace_kwargs=None):
    x = np.asarray(x, dtype=np.float32)
    weight_mu = np.asarray(weight_mu, dtype=np.float32)
    weight_rho = np.asarray(weight_rho, dtype=np.float32)
    bias_mu = np.asarray(bias_mu, dtype=np.float32)
    bias_rho = np.asarray(bias_rho, dtype=np.float32)
    eps_w = np.asarray(eps_w, dtype=np.float32)
    eps_b = np.asarray(eps_b, dtype=np.float32)

    nc = _get_nc()

    # k-major super-tile slabs: [16, 4096 (k), 1024 (n)] -> [65536, 1024]
    xb = x.astype(np.float16)
    xtv = np.ascontiguousarray(
        xb.reshape(NSUP, NB, IN_F).transpose(0, 2, 1)).reshape(NSUP * IN_F, NB)

    bias_full = bias_mu + np.log1p(np.exp(bias_rho)) * eps_b

    in_maps = []
    for c in range(N_CORES):
        osl = slice(c * OS, (c + 1) * OS)
        wc = np.concatenate([weight_rho[osl].astype(np.float16).T,
                             weight_mu[osl].astype(np.float16).T,
                             eps_w[osl].astype(np.float16).T], axis=1)
        in_maps.append({
            "xt": xtv,
            "wcat": np.ascontiguousarray(wc),
            "bias": np.ascontiguousarray(
                np.broadcast_to(bias_full[osl], (128, OS))),
        })

    kwargs = {}
    if _trace:
        kwargs["trace"] = True
        if _trace_kwargs:
            kwargs.update(_trace_kwargs)
    res = bass_utils.run_bass_kernel_spmd(
        nc, in_maps, core_ids=list(range(N_CORES)), **kwargs)

    out = np.empty((N, OUT_F), np.float32)
    for c in range(N_CORES):
        out[:, c * OS:(c + 1) * OS] = res.results[c]["out"]
    if _trace:
        return out, res
    return out
